# revision 17
# baseline (speedup 1.0000x reference)
"""Causal single-head attention on 8 Trainium2 NeuronCores (Bass/Tile).

Problem: x[4,2048,1024] fp32, Wq/Wk/Wv[1024,1024];
  q,k,v = x@W.T ; S = q@k.T/sqrt(d) ; causal softmax ; out = P@v.

Sharding: core c -> (batch b=c//2, query-half h=c%2): 1024 queries each.
K/V are computed per-core for the full batch sequence (duplicated across the
pair of cores sharing a batch) - no collectives.

SPMD uniformity trick for the causal mask: the host rotates each core's key
rows so its own query rows are always rows [0:1024) of the per-core input and
the causal triangle always falls in key-slots [0:1024). Slots [1024:2048) are
then either fully visible (h=1) or fully masked (h=0), selected by a per-core
scalar m1 in {0, -1e30} applied as data. The device program is identical on
all cores; only tensor contents differ.

All matmuls run as float32r (full-precision fp32 through the PE transpose-mode
datapath: 1 cycle/row at free-dim>=256 vs 4 cycles/row for plain fp32).
"""

import os
import sys

sys.path.insert(0, "/opt/trn_rl_repo")

from contextlib import ExitStack

import numpy as np

import concourse.bass as bass
from concourse import bacc
import concourse.mybir as mybir
import concourse.tile as tile
from concourse.bass_utils import run_bass_kernel_spmd

F32 = mybir.dt.float32
F32R = mybir.dt.float32r

B, N, D = 4, 2048, 1024
P = 128          # partition block
NQ = N // 2      # local queries per core (1024)
ND = D // P      # 8 d-blocks
NO = D // P      # 8 o-blocks
NS = N // P      # 16 key-slot blocks
NKC = N // 512   # 4 key chunks of 512
MASK_VAL = -1.0e30

_CACHE = {}


def _build_program(iters=1):
    nc = bacc.Bacc("TRN2", target_bir_lowering=False, debug=False, num_devices=8)
    xT = nc.dram_tensor("xT", [D, N], F32R, kind="ExternalInput").ap()
    wqT = nc.dram_tensor("wqT", [D, D], F32R, kind="ExternalInput").ap()
    wkT = nc.dram_tensor("wkT", [D, D], F32R, kind="ExternalInput").ap()
    wvT = nc.dram_tensor("wvT", [D, D], F32R, kind="ExternalInput").ap()
    m1 = nc.dram_tensor("m1", [P, 1], F32, kind="ExternalInput").ap()
    ident_d = nc.dram_tensor("ident", [P, P], F32R, kind="ExternalInput").ap()
    out = nc.dram_tensor("out", [NQ, D], F32, kind="ExternalOutput").ap()
    v_spill = nc.dram_tensor("v_spill", [N, D], F32R).ap()

    with tile.TileContext(nc) as tc:
        for _ in range(iters):
            _attention_kernel(tc, out, xT, wqT, wkT, wvT, m1, ident_d, v_spill)
    nc.compile()
    return nc


def _attention_kernel(tc, out, xT, wqT, wkT, wvT, m1, ident_d, v_spill):
    nc = tc.nc

    with ExitStack() as ctx:
        # ---- persistent pools ----
        const_pool = ctx.enter_context(tc.tile_pool(name="const", bufs=1))
        kt_pool = ctx.enter_context(tc.tile_pool(name="kt", bufs=1))
        qt_pool = ctx.enter_context(tc.tile_pool(name="qt", bufs=1))

        ident = const_pool.tile([P, P], F32, tag="ident")
        nc.gpsimd.dma_start(ident[:].bitcast(F32R), ident_d[:, :])
        m1_sb = const_pool.tile([P, 1], F32, tag="m1")
        nc.gpsimd.dma_start(m1_sb[:], m1[:, :])

        # KT[o-block][128, 2048] : k-projection output, o on partitions
        KT = [kt_pool.tile([P, N], F32, tag=f"kt{ob}", name=f"kt{ob}") for ob in range(NO)]
        # QT[o-block][128, 1024] : q-projection output, o on partitions
        QT = [qt_pool.tile([P, NQ], F32, tag=f"qt{ob}", name=f"qt{ob}") for ob in range(NO)]

        # ================= projections =================
        with ExitStack() as pctx:
            x_pool = pctx.enter_context(tc.tile_pool(name="xh", bufs=1))
            kvctx = pctx.enter_context(ExitStack())
            w_pool = kvctx.enter_context(tc.tile_pool(name="wfull", bufs=1))
            vtmp_pool = kvctx.enter_context(tc.tile_pool(name="vtmp", bufs=3))
            psum_kv = kvctx.enter_context(
                tc.tile_pool(name="psum_kv", bufs=8, space="PSUM")
            )

            # two passes over slot-halves; one x-half + one weight resident
            for half in (0, 1):
                xh = [
                    x_pool.tile([P, NQ], F32, tag=f"xh{d}", name=f"x{half}_{d}")
                    for d in range(ND)
                ]
                for d in range(ND):
                    nc.gpsimd.dma_start(
                        xh[d][:].bitcast(F32R),
                        xT[d * P : (d + 1) * P, half * NQ : (half + 1) * NQ],
                    )

                # --- K projection for this half ---
                wk = [
                    w_pool.tile([P, D], F32, tag=f"w{d}", name=f"wk{half}_{d}")
                    for d in range(ND)
                ]
                for d in range(ND):
                    nc.gpsimd.dma_start(wk[d][:].bitcast(F32R), wkT[d * P : (d + 1) * P, :])
                for ob in range(NO):
                    for kc in range(2):  # two 512-chunks within this half
                        ps = psum_kv.tile([P, 512], F32, tag="pskv")
                        for d in range(ND):
                            nc.tensor.matmul(
                                ps[:],
                                wk[d][:, ob * P : (ob + 1) * P].bitcast(F32R),
                                xh[d][:, kc * 512 : (kc + 1) * 512].bitcast(F32R),
                                start=(d == 0),
                                stop=(d == ND - 1),
                            )
                        col0 = half * NQ + kc * 512
                        nc.scalar.copy(KT[ob][:, col0 : col0 + 512].bitcast(F32R), ps[:])

                # --- V projection for this half (spilled to DRAM) ---
                wv = [
                    w_pool.tile([P, D], F32, tag=f"w{d}", name=f"wv{half}_{d}")
                    for d in range(ND)
                ]
                for d in range(ND):
                    nc.gpsimd.dma_start(wv[d][:].bitcast(F32R), wvT[d * P : (d + 1) * P, :])
                for sblk in range(8):  # slot blocks within this half
                    sb = half * 8 + sblk
                    vt = vtmp_pool.tile([P, D], F32, tag="vtmp")
                    for oc in range(2):
                        ps = psum_kv.tile([P, 512], F32, tag="pskv")
                        for d in range(ND):
                            nc.tensor.matmul(
                                ps[:],
                                xh[d][:, sblk * P : (sblk + 1) * P].bitcast(F32R),
                                wv[d][:, oc * 512 : (oc + 1) * 512].bitcast(F32R),
                                start=(d == 0),
                                stop=(d == ND - 1),
                            )
                        nc.scalar.copy(vt[:, oc * 512 : (oc + 1) * 512].bitcast(F32R), ps[:])
                    nc.gpsimd.dma_start(v_spill[sb * P : (sb + 1) * P, :], vt[:].bitcast(F32R))

            kvctx.close()  # release w/vtmp/psum_kv before Q-projection pools

            # --- Q projection (re-streams x half 0; streams wq per d-block) ---
            with ExitStack() as qctx:
                wq_pool = qctx.enter_context(tc.tile_pool(name="wqs", bufs=3))
                psum_q = qctx.enter_context(
                    tc.tile_pool(name="psum_q", bufs=1, space="PSUM")
                )
                xq = [
                    x_pool.tile([P, NQ], F32, tag=f"xh{d}", name=f"xq{d}")
                    for d in range(ND)
                ]
                for d in range(ND):
                    nc.gpsimd.dma_start(xq[d][:].bitcast(F32R), xT[d * P : (d + 1) * P, 0:NQ])
                for qpass in range(2):  # o-blocks 0-3, then 4-7
                    qp = [
                        psum_q.tile([P, 512], F32, tag=f"psq{i}", name=f"psq{i}")
                        for i in range(8)
                    ]
                    for d in range(ND):
                        wq = wq_pool.tile([P, 512], F32, tag="wqs")
                        nc.gpsimd.dma_start(
                            wq[:].bitcast(F32R),
                            wqT[d * P : (d + 1) * P, qpass * 512 : (qpass + 1) * 512],
                        )
                        for obi in range(4):
                            for qc in range(2):
                                nc.tensor.matmul(
                                    qp[obi * 2 + qc][:],
                                    wq[:, obi * P : (obi + 1) * P].bitcast(F32R),
                                    xq[d][:, qc * 512 : (qc + 1) * 512].bitcast(F32R),
                                    start=(d == 0),
                                    stop=(d == ND - 1),
                                )
                    for obi in range(4):
                        ob = qpass * 4 + obi
                        for qc in range(2):
                            nc.scalar.copy(
                                QT[ob][:, qc * 512 : (qc + 1) * 512].bitcast(F32R),
                                qp[obi * 2 + qc][:],
                            )

        # ================= attention =================
        with ExitStack() as actx:
            s_pool = actx.enter_context(tc.tile_pool(name="s", bufs=3))
            stat_pool = actx.enter_context(tc.tile_pool(name="stat", bufs=4))
            pt_pool = actx.enter_context(tc.tile_pool(name="pt", bufs=3))
            o_pool = actx.enter_context(tc.tile_pool(name="o", bufs=2))
            vin_pool = actx.enter_context(tc.tile_pool(name="vin", bufs=4))
            psum_s = actx.enter_context(tc.tile_pool(name="psum_s", bufs=4, space="PSUM"))
            psum_t = actx.enter_context(tc.tile_pool(name="psum_t", bufs=2, space="PSUM"))
            psum_o = actx.enter_context(tc.tile_pool(name="psum_o", bufs=2, space="PSUM"))

            for qb in range(NQ // P):  # 8 query blocks
                S = s_pool.tile([P, N], F32, tag="s")
                # scores: S[q, slot] = sum_o QT[o, q] * KT[o, slot]
                for kc in range(NKC):
                    ps = psum_s.tile([P, 512], F32, tag="pss")
                    for ob in range(NO):
                        nc.tensor.matmul(
                            ps[:],
                            QT[ob][:, qb * P : (qb + 1) * P].bitcast(F32R),
                            KT[ob][:, kc * 512 : (kc + 1) * 512].bitcast(F32R),
                            start=(ob == 0),
                            stop=(ob == NO - 1),
                        )
                    if kc < 2:
                        # causal-triangle half: plain copy (mask applied below)
                        nc.vector.tensor_copy(S[:, kc * 512 : (kc + 1) * 512].bitcast(F32R), ps[:])
                    else:
                        # far half: fully visible (h=1) or fully masked (h=0)
                        nc.vector.tensor_scalar_add(
                            S[:, kc * 512 : (kc + 1) * 512].bitcast(F32R),
                            ps[:],
                            m1_sb[:],
                        )
                # causal mask on slots [0:1024): keep iff (qb*128 + p - f) >= 0
                nc.gpsimd.affine_select(
                    out=S[:, 0:NQ].bitcast(F32R),
                    in_=S[:, 0:NQ],
                    compare_op=mybir.AluOpType.is_ge,
                    fill=MASK_VAL,
                    base=qb * P,
                    pattern=[[-1, NQ]],
                    channel_multiplier=1,
                )
                neg_max = stat_pool.tile([P, 1], F32, tag="negmax")
                nc.vector.reduce_max(
                    neg_max[:], S[:], axis=mybir.AxisListType.X, negate=True
                )
                zrow = stat_pool.tile([P, 1], F32, tag="zrow")
                nc.scalar.activation(
                    S[:].bitcast(F32R),
                    S[:],
                    mybir.ActivationFunctionType.Exp,
                    bias=neg_max[:],
                    scale=1.0,
                    accum_out=zrow[:],
                )
                rz = stat_pool.tile([P, 1], F32, tag="rz")
                nc.vector.reciprocal(rz[:], zrow[:])

                # AV: O[q, o] = sum_slots P[q, slot] V[slot, o]
                op0 = psum_o.tile([P, 512], F32, tag="pso", name="op0")
                op1 = psum_o.tile([P, 512], F32, tag="pso", name="op1")
                for sb in range(NS):
                    vin = vin_pool.tile([P, D], F32, tag="vin")
                    nc.gpsimd.dma_start(vin[:].bitcast(F32R), v_spill[sb * P : (sb + 1) * P, :])
                    tp = psum_t.tile([P, P], F32, tag="pst")
                    nc.tensor.transpose(
                        tp[:].bitcast(F32R),
                        S[:, sb * P : (sb + 1) * P].bitcast(F32R),
                        ident[:].bitcast(F32R),
                    )
                    pt = pt_pool.tile([P, P], F32, tag="pt")
                    nc.vector.tensor_copy(pt[:].bitcast(F32R), tp[:])
                    for oc, op in ((0, op0), (1, op1)):
                        nc.tensor.matmul(
                            op[:],
                            pt[:].bitcast(F32R),
                            vin[:, oc * 512 : (oc + 1) * 512].bitcast(F32R),
                            start=(sb == 0),
                            stop=(sb == NS - 1),
                        )
                O = o_pool.tile([P, D], F32, tag="o")
                nc.vector.tensor_scalar_mul(O[:, 0:512], op0[:], rz[:])
                nc.vector.tensor_scalar_mul(O[:, 512:1024], op1[:], rz[:])
                nc.gpsimd.dma_start(out[qb * P : (qb + 1) * P, :], O[:])


def _get_program(iters=1):
    key = ("nc", iters)
    if key not in _CACHE:
        _CACHE[key] = _build_program(iters)
    return _CACHE[key]


def _host_prep(x, Wq, Wk, Wv):
    scale = np.float32(1.0 / np.sqrt(np.float32(D)))
    wqT = np.ascontiguousarray((np.asarray(Wq, np.float32) * scale).T)
    wkT = np.ascontiguousarray(np.asarray(Wk, np.float32).T)
    wvT = np.ascontiguousarray(np.asarray(Wv, np.float32).T)
    in_maps = []
    for c in range(8):
        b, h = c // 2, c % 2
        xb_ = np.asarray(x[b], dtype=np.float32)
        if h == 0:
            xrot = xb_
            m1v = MASK_VAL
        else:
            xrot = np.concatenate([xb_[NQ:], xb_[:NQ]], axis=0)
            m1v = 0.0
        in_maps.append(
            {
                "xT": np.ascontiguousarray(xrot.T),
                "wqT": wqT,
                "wkT": wkT,
                "wvT": wvT,
                "m1": np.full((P, 1), m1v, np.float32),
                "ident": np.eye(P, dtype=np.float32),
            }
        )
    return in_maps


def kernel(x, Wq, Wk, Wv):
    nc = _get_program()
    in_maps = _host_prep(x, Wq, Wk, Wv)
    res = run_bass_kernel_spmd(nc, in_maps, list(range(8)))
    _CACHE["last_results"] = res
    out = np.empty((B, N, D), np.float32)
    for c in range(8):
        b, h = c // 2, c % 2
        out[b, h * NQ : (h + 1) * NQ] = res.results[c]["out"]
    return out


# revision 20
# speedup vs baseline: 2.4374x; 2.4374x over previous
"""Causal single-head attention on 8 Trainium2 NeuronCores (Bass/Tile).

Problem: x[4,2048,1024] fp32, Wq/Wk/Wv[1024,1024];
  q,k,v = x@W.T ; S = q@k.T/sqrt(d) ; causal softmax ; out = P@v.

Sharding: core c -> (batch b=c//2, query-half h=c%2): 1024 queries each.
K/V are computed per-core for the full batch sequence (duplicated across the
pair of cores sharing a batch) - no collectives.

SPMD uniformity trick for the causal mask: the host rotates each core's key
rows so its own query rows are always rows [0:1024) of the per-core input and
the causal triangle always falls in key-slots [0:1024). Slots [1024:2048) are
then either fully visible (h=1) or fully masked (h=0), selected by a per-core
scalar m1 in {0, -1e30} applied as data. The device program is identical on
all cores; only tensor contents differ.

All matmuls run as float32r (full-precision fp32 through the PE transpose-mode
datapath: 1 cycle/row at free-dim>=256 vs 4 cycles/row for plain fp32).
"""

import os
import sys

sys.path.insert(0, "/opt/trn_rl_repo")

from contextlib import ExitStack

import numpy as np

import concourse.bass as bass
from concourse import bacc
import concourse.mybir as mybir
import concourse.tile as tile
from concourse.bass_utils import run_bass_kernel_spmd

F32 = mybir.dt.float32
F32R = mybir.dt.float32r

B, N, D = 4, 2048, 1024
P = 128          # partition block
NQ = N // 2      # local queries per core (1024)
ND = D // P      # 8 d-blocks
NO = D // P      # 8 o-blocks
NS = N // P      # 16 key-slot blocks
NKC = N // 512   # 4 key chunks of 512
MASK_VAL = -1.0e30

_CACHE = {}


def _build_program(iters=1):
    nc = bacc.Bacc("TRN2", target_bir_lowering=False, debug=False, num_devices=8)
    xT = nc.dram_tensor("xT", [D, N], F32R, kind="ExternalInput").ap()
    wqT = nc.dram_tensor("wqT", [D, D], F32R, kind="ExternalInput").ap()
    wkT = nc.dram_tensor("wkT", [D, D], F32R, kind="ExternalInput").ap()
    wvT = nc.dram_tensor("wvT", [D, D], F32R, kind="ExternalInput").ap()
    m1 = nc.dram_tensor("m1", [P, 1], F32, kind="ExternalInput").ap()
    ident_d = nc.dram_tensor("ident", [P, P], F32R, kind="ExternalInput").ap()
    out = nc.dram_tensor("out", [NQ, D], F32, kind="ExternalOutput").ap()

    with tile.TileContext(nc) as tc:
        for _ in range(iters):
            _attention_kernel(tc, out, xT, wqT, wkT, wvT, m1, ident_d)
    nc.compile()
    return nc


def _attention_kernel(tc, out, xT, wqT, wkT, wvT, m1, ident_d):
    nc = tc.nc

    with ExitStack() as ctx:
        # ---- persistent pools ----
        const_pool = ctx.enter_context(tc.tile_pool(name="const", bufs=1))
        kt_pool = ctx.enter_context(tc.tile_pool(name="kt", bufs=1))
        v_pool = ctx.enter_context(tc.tile_pool(name="v", bufs=1))

        ident = const_pool.tile([P, P], F32, tag="ident")
        nc.sync.dma_start(ident[:].bitcast(F32R), ident_d[:, :])
        m1_sb = const_pool.tile([P, 1], F32, tag="m1")
        nc.sync.dma_start(m1_sb[:], m1[:, :])

        # KT[o-block][128, 2048] : k-projection output, o on partitions
        KT = [kt_pool.tile([P, N], F32, tag=f"kt{ob}", name=f"kt{ob}") for ob in range(NO)]
        # V[slot-block][128, 1024] : v-projection output, slots on partitions
        V = [v_pool.tile([P, D], F32, tag=f"v{sb}", name=f"v{sb}") for sb in range(NS)]

        # ================= projections =================
        with ExitStack() as pctx:
            x_pool = pctx.enter_context(tc.tile_pool(name="xh", bufs=1))
            kvctx = pctx.enter_context(ExitStack())
            w_pool = kvctx.enter_context(tc.tile_pool(name="wfull", bufs=1))
            psum_kv = kvctx.enter_context(
                tc.tile_pool(name="psum_kv", bufs=8, space="PSUM")
            )

            # half 1 first so half 0 (the query rows) stays resident for Q-proj
            for half in (1, 0):
                xh = [
                    x_pool.tile([P, NQ], F32, tag=f"xh{d}", name=f"x{half}_{d}")
                    for d in range(ND)
                ]
                for d in range(ND):
                    nc.sync.dma_start(
                        xh[d][:].bitcast(F32R),
                        xT[d * P : (d + 1) * P, half * NQ : (half + 1) * NQ],
                    )

                # --- K projection for this half ---
                wk = [
                    w_pool.tile([P, D], F32, tag=f"w{d}", name=f"wk{half}_{d}")
                    for d in range(ND)
                ]
                for d in range(ND):
                    nc.sync.dma_start(wk[d][:].bitcast(F32R), wkT[d * P : (d + 1) * P, :])
                for ob in range(NO):
                    for kc in range(2):  # two 512-chunks within this half
                        ps = psum_kv.tile([P, 512], F32, tag="pskv")
                        for d in range(ND):
                            nc.tensor.matmul(
                                ps[:],
                                wk[d][:, ob * P : (ob + 1) * P].bitcast(F32R),
                                xh[d][:, kc * 512 : (kc + 1) * 512].bitcast(F32R),
                                start=(d == 0),
                                stop=(d == ND - 1),
                            )
                        col0 = half * NQ + kc * 512
                        nc.scalar.copy(KT[ob][:, col0 : col0 + 512].bitcast(F32R), ps[:])

                # --- V projection for this half (direct into resident V) ---
                wv = [
                    w_pool.tile([P, D], F32, tag=f"w{d}", name=f"wv{half}_{d}")
                    for d in range(ND)
                ]
                for d in range(ND):
                    nc.sync.dma_start(wv[d][:].bitcast(F32R), wvT[d * P : (d + 1) * P, :])
                for sblk in range(8):  # slot blocks within this half
                    sb = half * 8 + sblk
                    for oc in range(2):
                        ps = psum_kv.tile([P, 512], F32, tag="pskv")
                        for d in range(ND):
                            nc.tensor.matmul(
                                ps[:],
                                xh[d][:, sblk * P : (sblk + 1) * P].bitcast(F32R),
                                wv[d][:, oc * 512 : (oc + 1) * 512].bitcast(F32R),
                                start=(d == 0),
                                stop=(d == ND - 1),
                            )
                        nc.scalar.copy(
                            V[sb][:, oc * 512 : (oc + 1) * 512].bitcast(F32R), ps[:]
                        )

            kvctx.close()  # release wfull/psum_kv before QT + Q-projection pools

            # QT outlives the projection phase - allocated on the outer stack
            # after the weight pool is freed so the peak stays under budget.
            qt_pool = ctx.enter_context(tc.tile_pool(name="qt", bufs=1, side="right"))
            QT = [
                qt_pool.tile([P, NQ], F32, tag=f"qt{ob}", name=f"qt{ob}")
                for ob in range(NO)
            ]

            # --- Q projection (x half 0 resident; streams wq per d-block) ---
            with ExitStack() as qctx:
                wq_pool = qctx.enter_context(tc.tile_pool(name="wqs", bufs=3))
                psum_q = qctx.enter_context(
                    tc.tile_pool(name="psum_q", bufs=1, space="PSUM")
                )
                for qpass in range(2):  # o-blocks 0-3, then 4-7
                    qp = [
                        psum_q.tile([P, 512], F32, tag=f"psq{i}", name=f"psq{i}")
                        for i in range(8)
                    ]
                    for d in range(ND):
                        wq = wq_pool.tile([P, 512], F32, tag="wqs")
                        nc.sync.dma_start(
                            wq[:].bitcast(F32R),
                            wqT[d * P : (d + 1) * P, qpass * 512 : (qpass + 1) * 512],
                        )
                        for obi in range(4):
                            for qc in range(2):
                                nc.tensor.matmul(
                                    qp[obi * 2 + qc][:],
                                    wq[:, obi * P : (obi + 1) * P].bitcast(F32R),
                                    xh[d][:, qc * 512 : (qc + 1) * 512].bitcast(F32R),
                                    start=(d == 0),
                                    stop=(d == ND - 1),
                                )
                    for obi in range(4):
                        ob = qpass * 4 + obi
                        for qc in range(2):
                            nc.scalar.copy(
                                QT[ob][:, qc * 512 : (qc + 1) * 512].bitcast(F32R),
                                qp[obi * 2 + qc][:],
                            )

        # ================= attention =================
        with ExitStack() as actx:
            s_pool = actx.enter_context(tc.tile_pool(name="s", bufs=3))
            stat_pool = actx.enter_context(tc.tile_pool(name="stat", bufs=4))
            pt_pool = actx.enter_context(tc.tile_pool(name="pt", bufs=3))
            o_pool = actx.enter_context(tc.tile_pool(name="o", bufs=2))
            psum_s = actx.enter_context(tc.tile_pool(name="psum_s", bufs=4, space="PSUM"))
            psum_t = actx.enter_context(tc.tile_pool(name="psum_t", bufs=2, space="PSUM"))
            psum_o = actx.enter_context(tc.tile_pool(name="psum_o", bufs=2, space="PSUM"))

            for qb in range(NQ // P):  # 8 query blocks
                S = s_pool.tile([P, N], F32, tag="s")
                # scores: S[q, slot] = sum_o QT[o, q] * KT[o, slot]
                for kc in range(NKC):
                    ps = psum_s.tile([P, 512], F32, tag="pss")
                    for ob in range(NO):
                        nc.tensor.matmul(
                            ps[:],
                            QT[ob][:, qb * P : (qb + 1) * P].bitcast(F32R),
                            KT[ob][:, kc * 512 : (kc + 1) * 512].bitcast(F32R),
                            start=(ob == 0),
                            stop=(ob == NO - 1),
                        )
                    if kc < 2:
                        # causal-triangle half: plain copy (mask applied below)
                        nc.vector.tensor_copy(
                            S[:, kc * 512 : (kc + 1) * 512].bitcast(F32R), ps[:]
                        )
                    else:
                        # far half: fully visible (h=1) or fully masked (h=0)
                        nc.vector.tensor_scalar_add(
                            S[:, kc * 512 : (kc + 1) * 512].bitcast(F32R),
                            ps[:],
                            m1_sb[:],
                        )
                # causal mask on slots [0:1024): keep iff (qb*128 + p - f) >= 0
                nc.gpsimd.affine_select(
                    out=S[:, 0:NQ].bitcast(F32R),
                    in_=S[:, 0:NQ],
                    compare_op=mybir.AluOpType.is_ge,
                    fill=MASK_VAL,
                    base=qb * P,
                    pattern=[[-1, NQ]],
                    channel_multiplier=1,
                )
                neg_max = stat_pool.tile([P, 1], F32, tag="negmax")
                nc.vector.reduce_max(
                    neg_max[:], S[:], axis=mybir.AxisListType.X, negate=True
                )
                zrow = stat_pool.tile([P, 1], F32, tag="zrow")
                nc.scalar.activation(
                    S[:].bitcast(F32R),
                    S[:],
                    mybir.ActivationFunctionType.Exp,
                    bias=neg_max[:],
                    scale=1.0,
                    accum_out=zrow[:],
                )
                rz = stat_pool.tile([P, 1], F32, tag="rz")
                nc.vector.reciprocal(rz[:], zrow[:])

                # AV: O[q, o] = sum_slots P[q, slot] V[slot, o]
                op0 = psum_o.tile([P, 512], F32, tag="pso", name="op0")
                op1 = psum_o.tile([P, 512], F32, tag="pso", name="op1")
                for sb in range(NS):
                    tp = psum_t.tile([P, P], F32, tag="pst")
                    nc.tensor.transpose(
                        tp[:].bitcast(F32R),
                        S[:, sb * P : (sb + 1) * P].bitcast(F32R),
                        ident[:].bitcast(F32R),
                    )
                    pt = pt_pool.tile([P, P], F32, tag="pt")
                    nc.vector.tensor_copy(pt[:].bitcast(F32R), tp[:])
                    for oc, op in ((0, op0), (1, op1)):
                        nc.tensor.matmul(
                            op[:],
                            pt[:].bitcast(F32R),
                            V[sb][:, oc * 512 : (oc + 1) * 512].bitcast(F32R),
                            start=(sb == 0),
                            stop=(sb == NS - 1),
                        )
                O = o_pool.tile([P, D], F32, tag="o")
                nc.vector.tensor_scalar_mul(O[:, 0:512], op0[:], rz[:])
                nc.vector.tensor_scalar_mul(O[:, 512:1024], op1[:], rz[:])
                nc.sync.dma_start(out[qb * P : (qb + 1) * P, :], O[:])


def _get_program(iters=1):
    key = ("nc", iters)
    if key not in _CACHE:
        _CACHE[key] = _build_program(iters)
    return _CACHE[key]


def _host_prep(x, Wq, Wk, Wv):
    scale = np.float32(1.0 / np.sqrt(np.float32(D)))
    wqT = np.ascontiguousarray((np.asarray(Wq, np.float32) * scale).T)
    wkT = np.ascontiguousarray(np.asarray(Wk, np.float32).T)
    wvT = np.ascontiguousarray(np.asarray(Wv, np.float32).T)
    in_maps = []
    for c in range(8):
        b, h = c // 2, c % 2
        xb_ = np.asarray(x[b], dtype=np.float32)
        if h == 0:
            xrot = xb_
            m1v = MASK_VAL
        else:
            xrot = np.concatenate([xb_[NQ:], xb_[:NQ]], axis=0)
            m1v = 0.0
        in_maps.append(
            {
                "xT": np.ascontiguousarray(xrot.T),
                "wqT": wqT,
                "wkT": wkT,
                "wvT": wvT,
                "m1": np.full((P, 1), m1v, np.float32),
                "ident": np.eye(P, dtype=np.float32),
            }
        )
    return in_maps


def kernel(x, Wq, Wk, Wv):
    nc = _get_program()
    in_maps = _host_prep(x, Wq, Wk, Wv)
    res = run_bass_kernel_spmd(nc, in_maps, list(range(8)))
    _CACHE["last_results"] = res
    out = np.empty((B, N, D), np.float32)
    for c in range(8):
        b, h = c // 2, c % 2
        out[b, h * NQ : (h + 1) * NQ] = res.results[c]["out"]
    return out
